# revision 8
# baseline (speedup 1.0000x reference)
"""CGConv x2 GNN message-passing kernel for Trainium2 (8 NeuronCores).

Strategy:
  - Factor z@W GEMMs into per-node projections H = x @ W_node  (4x FLOP cut):
      z @ Wf = x[dst] @ Wf[0:128] + x[src] @ Wf[128:256] + e @ Wf[256:320]
  - Sort edges by dst on host; 128-node windows; contiguous window ranges
    sharded across 8 cores => scatter-mean is core-local (no all-reduce).
  - Per 128-edge chunk: edge-attr GEMM (bias folded as K=65 row), indirect-DMA
    gathers of node projections, sigmoid*softplus on ACT, one-hot scatter
    matmul accumulating into a per-window PSUM tile.
  - BN (eval) folded: scale gamma' into a broadcast multiplier, shift b' into
    the residual input (host-side).
  - One compiled NEFF, executed twice (layer 1, then layer 2 after a host
    gather of the transposed layer-1 outputs).
"""

import numpy as np
import ml_dtypes

import concourse.bass as bass
import concourse.tile as tile
from concourse import bacc, mybir
from concourse.bass_utils import run_bass_kernel_spmd
from concourse.masks import make_identity

P = 128
N_CORES = 8
F_NODE = 128
F_EDGE = 64
KE = F_EDGE + 1  # edge GEMM contraction with bias row
BN_EPS = 1e-5

BF16 = mybir.dt.bfloat16
F32 = mybir.dt.float32
I32 = mybir.dt.int32
NP_BF16 = ml_dtypes.bfloat16

# populated by kernel() for test.py
LAST_EXEC_NS = []
LAST_WALL_S = []


# ---------------------------------------------------------------- device code

def build_nc(W_PER_CORE, C, NLOC, N_PAD):
    """Build+compile the per-layer SPMD program.

    W_PER_CORE windows x C chunks x 128 edges per core. NLOC = W_PER_CORE*128
    local nodes, N_PAD = 8*NLOC global padded nodes.
    """
    E_PAD = W_PER_CORE * C * P
    NCHUNK_COLS = W_PER_CORE * C
    NHCHUNK = N_PAD // P  # H-GEMM node chunks (all cores compute full table)
    NHLOC = NLOC // P     # first NHLOC chunks also write the local dst table

    nc = bacc.Bacc("TRN2", target_bir_lowering=False, debug=False,
                   num_devices=N_CORES)

    # inputs
    xT = nc.dram_tensor("xT", [P, N_PAD], F32, kind="ExternalInput")
    w_all = nc.dram_tensor("w_all", [P, 4 * F_NODE], F32, kind="ExternalInput")
    w_ext = nc.dram_tensor("w_ext", [KE, 2 * F_NODE], F32, kind="ExternalInput")
    esortT = nc.dram_tensor("esortT", [KE, E_PAD], F32, kind="ExternalInput")
    srcidx = nc.dram_tensor("srcidx", [P, NCHUNK_COLS], I32, kind="ExternalInput")
    dstidx = nc.dram_tensor("dstidx", [P, NCHUNK_COLS], I32, kind="ExternalInput")
    dstloc = nc.dram_tensor("dstloc", [P, NCHUNK_COLS], BF16, kind="ExternalInput")
    invcnt = nc.dram_tensor("invcnt", [P, W_PER_CORE], F32, kind="ExternalInput")
    xr = nc.dram_tensor("xr", [NLOC, F_NODE], F32, kind="ExternalInput")
    gbc = nc.dram_tensor("gbc", [P, F_NODE], F32, kind="ExternalInput")

    # outputs
    out = nc.dram_tensor("out", [NLOC, F_NODE], F32, kind="ExternalOutput")
    outT = nc.dram_tensor("outT", [P, NLOC], F32, kind="ExternalOutput")

    with tile.TileContext(nc) as tc:
        with (
            tc.tile_pool(name="const", bufs=1) as cpool,
            tc.tile_pool(name="hx", bufs=3) as hxp,
            tc.tile_pool(name="hev", bufs=3) as hevp,
            tc.tile_pool(name="es", bufs=2) as esp,
            tc.tile_pool(name="widx", bufs=2) as widxp,
            tc.tile_pool(name="gath", bufs=4) as gathp,
            tc.tile_pool(name="ew", bufs=4) as ewp,
            tc.tile_pool(name="fin", bufs=2) as finp,
            tc.tile_pool(name="hps", bufs=2, space="PSUM") as hps,
            tc.tile_pool(name="eps", bufs=2, space="PSUM") as epsp,
            tc.tile_pool(name="wps", bufs=2, space="PSUM") as wpsp,
            tc.tile_pool(name="tps", bufs=2, space="PSUM") as tpsp,
            tc.tile_pool(name="dram", bufs=1, space="DRAM") as dramp,
        ):
            # node projection tables (bf16)
            hd_tab = dramp.tile([NLOC, 2 * F_NODE], BF16)
            hs_tab = dramp.tile([N_PAD, 2 * F_NODE], BF16)

            # constants
            w_all_t = cpool.tile([P, 4 * F_NODE], F32)
            nc.sync.dma_start(out=w_all_t[:], in_=w_all[:, :])
            w_ext_t = cpool.tile([KE, 2 * F_NODE], F32)
            nc.sync.dma_start(out=w_ext_t[:], in_=w_ext[:, :])
            gbc_t = cpool.tile([P, F_NODE], F32)
            nc.sync.dma_start(out=gbc_t[:], in_=gbc[:, :])
            inv_t = cpool.tile([P, W_PER_CORE], F32)
            nc.sync.dma_start(out=inv_t[:], in_=invcnt[:, :])
            ident = cpool.tile([P, P], F32)
            make_identity(nc, ident[:])
            iota_i = cpool.tile([P, P], I32)
            nc.gpsimd.iota(iota_i[:], pattern=[[1, P]], base=0, channel_multiplier=0)
            iota_b = cpool.tile([P, P], BF16)
            nc.vector.tensor_copy(out=iota_b[:], in_=iota_i[:])

            # ---- phase 1: H = x @ W_all ----
            for j in range(NHCHUNK):
                xt = hxp.tile([P, P], F32)
                nc.sync.dma_start(out=xt[:], in_=xT[:, j * P:(j + 1) * P])
                hp = hps.tile([P, 4 * F_NODE], F32, space="PSUM")
                nc.tensor.matmul(out=hp[:], lhsT=xt[:], rhs=w_all_t[:],
                                 start=True, stop=True)
                hs_sb = hevp.tile([P, 2 * F_NODE], BF16, tag="hs_sb")
                nc.scalar.copy(out=hs_sb[:], in_=hp[:, 2 * F_NODE:4 * F_NODE])
                nc.sync.dma_start(out=hs_tab[j * P:(j + 1) * P, :], in_=hs_sb[:])
                if j < NHLOC:
                    hd_sb = hevp.tile([P, 2 * F_NODE], BF16, tag="hd_sb")
                    nc.vector.tensor_copy(out=hd_sb[:], in_=hp[:, 0:2 * F_NODE])
                    nc.sync.dma_start(out=hd_tab[j * P:(j + 1) * P, :], in_=hd_sb[:])

            # ---- phase 2: edges ----
            for w in range(W_PER_CORE):
                es_t = esp.tile([KE, C * P], F32)
                nc.sync.dma_start(out=es_t[:], in_=esortT[:, w * C * P:(w + 1) * C * P])
                si_t = widxp.tile([P, C], I32, tag="si")
                nc.sync.dma_start(out=si_t[:], in_=srcidx[:, w * C:(w + 1) * C])
                di_t = widxp.tile([P, C], I32, tag="di")
                nc.sync.dma_start(out=di_t[:], in_=dstidx[:, w * C:(w + 1) * C])
                dl_t = widxp.tile([P, C], BF16, tag="dl")
                nc.sync.dma_start(out=dl_t[:], in_=dstloc[:, w * C:(w + 1) * C])

                acc = wpsp.tile([P, F_NODE], F32, space="PSUM")
                for c in range(C):
                    hs_g = gathp.tile([P, 2 * F_NODE], BF16, tag="hs_g")
                    nc.gpsimd.indirect_dma_start(
                        out=hs_g[:], out_offset=None, in_=hs_tab[:, :],
                        in_offset=bass.IndirectOffsetOnAxis(ap=si_t[:, c:c + 1], axis=0))
                    hd_g = gathp.tile([P, 2 * F_NODE], BF16, tag="hd_g")
                    nc.gpsimd.indirect_dma_start(
                        out=hd_g[:], out_offset=None, in_=hd_tab[:, :],
                        in_offset=bass.IndirectOffsetOnAxis(ap=di_t[:, c:c + 1], axis=0))
                    hsum = ewp.tile([P, 2 * F_NODE], BF16, tag="hsum")
                    nc.vector.tensor_tensor(out=hsum[:], in0=hd_g[:], in1=hs_g[:],
                                            op=mybir.AluOpType.add)
                    ep = epsp.tile([P, 2 * F_NODE], F32, space="PSUM")
                    nc.tensor.matmul(out=ep[:], lhsT=es_t[:, c * P:(c + 1) * P],
                                     rhs=w_ext_t[:], start=True, stop=True)
                    arg = ewp.tile([P, 2 * F_NODE], F32, tag="arg")
                    nc.vector.scalar_tensor_tensor(
                        out=arg[:], in0=ep[:], scalar=1.0, in1=hsum[:],
                        op0=mybir.AluOpType.mult, op1=mybir.AluOpType.add)
                    # single act table (Exp/Ln/Copy):
                    #   softplus(b) = Ln(Exp(b)+1);  sigmoid(a) = 1/(1+Exp(-a))
                    ta = ewp.tile([P, F_NODE], F32, tag="ta")
                    nc.scalar.activation(out=ta[:], in_=arg[:, 0:F_NODE],
                                         func=mybir.ActivationFunctionType.Exp,
                                         scale=-1.0)
                    tb = ewp.tile([P, F_NODE], F32, tag="tb")
                    nc.scalar.activation(out=tb[:], in_=arg[:, F_NODE:2 * F_NODE],
                                         func=mybir.ActivationFunctionType.Exp)
                    sp = ewp.tile([P, F_NODE], F32, tag="sp")
                    nc.scalar.activation(out=sp[:], in_=tb[:],
                                         func=mybir.ActivationFunctionType.Ln,
                                         bias=1.0)
                    q = ewp.tile([P, F_NODE], F32, tag="q")
                    nc.vector.tensor_scalar_add(out=q[:], in0=ta[:], scalar1=1.0)
                    r = ewp.tile([P, F_NODE], F32, tag="r")
                    nc.vector.reciprocal(out=r[:], in_=q[:])
                    msg = ewp.tile([P, F_NODE], BF16, tag="msg")
                    nc.vector.tensor_tensor(out=msg[:], in0=sp[:], in1=r[:],
                                            op=mybir.AluOpType.mult)
                    oneh = ewp.tile([P, P], BF16, tag="oneh")
                    nc.vector.tensor_tensor(
                        out=oneh[:], in0=dl_t[:, c:c + 1].to_broadcast([P, P]),
                        in1=iota_b[:], op=mybir.AluOpType.is_equal)
                    nc.tensor.matmul(out=acc[:], lhsT=oneh[:], rhs=msg[:],
                                     start=(c == 0), stop=(c == C - 1))

                # finalize window: out = acc*inv*gamma' + xr
                xr_t = finp.tile([P, F_NODE], F32, tag="xr")
                nc.sync.dma_start(out=xr_t[:], in_=xr[w * P:(w + 1) * P, :])
                t1 = finp.tile([P, F_NODE], F32, tag="t1")
                nc.vector.scalar_tensor_tensor(
                    out=t1[:], in0=acc[:], scalar=inv_t[:, w:w + 1], in1=gbc_t[:],
                    op0=mybir.AluOpType.mult, op1=mybir.AluOpType.mult)
                o_t = finp.tile([P, F_NODE], F32, tag="o")
                nc.vector.tensor_tensor(out=o_t[:], in0=t1[:], in1=xr_t[:],
                                        op=mybir.AluOpType.add)
                nc.sync.dma_start(out=out[w * P:(w + 1) * P, :], in_=o_t[:])
                tp = tpsp.tile([P, P], F32, space="PSUM")
                nc.tensor.transpose(out=tp[:], in_=o_t[:], identity=ident[:])
                oT_t = finp.tile([P, P], F32, tag="oT")
                nc.scalar.copy(out=oT_t[:], in_=tp[:])
                nc.sync.dma_start(out=outT[:, w * P:(w + 1) * P], in_=oT_t[:])

    nc.compile()
    return nc


# ------------------------------------------------------------- host preprocess

def _fold_bn(gamma, beta, rmean, rvar):
    g = gamma / np.sqrt(rvar + BN_EPS)
    b = beta - rmean * g
    return g.astype(np.float32), b.astype(np.float32)


def _layer_weights(Wf, bf, Ws, bs, bias_shift=None):
    """W_all [128, 512], W_ext [65, 256] with bias row (minus optional shift)."""
    Wf = np.asarray(Wf, np.float32)
    Ws = np.asarray(Ws, np.float32)
    w_all = np.concatenate(
        [Wf[0:F_NODE], Ws[0:F_NODE], Wf[F_NODE:2 * F_NODE], Ws[F_NODE:2 * F_NODE]],
        axis=1).astype(np.float32)  # [128, 512] cols: Hf_d|Hs_d|Hf_s|Hs_s
    bf_eff = np.asarray(bf, np.float32).copy()
    bs_eff = np.asarray(bs, np.float32).copy()
    if bias_shift is not None:
        bf_eff -= bias_shift @ (Wf[0:F_NODE] + Wf[F_NODE:2 * F_NODE])
        bs_eff -= bias_shift @ (Ws[0:F_NODE] + Ws[F_NODE:2 * F_NODE])
    w_ext = np.zeros((KE, 2 * F_NODE), np.float32)
    w_ext[0:F_EDGE, 0:F_NODE] = Wf[2 * F_NODE:]
    w_ext[0:F_EDGE, F_NODE:] = Ws[2 * F_NODE:]
    w_ext[F_EDGE, 0:F_NODE] = bf_eff
    w_ext[F_EDGE, F_NODE:] = bs_eff
    return w_all, w_ext


def preprocess_graph(edge_index, edge_attr, n_nodes):
    """Static per-core arrays from the graph structure."""
    src = np.asarray(edge_index[0]).astype(np.int64)
    dst = np.asarray(edge_index[1]).astype(np.int64)
    ea = np.asarray(edge_attr, np.float32)

    n_win = -(-n_nodes // P)
    W_PER_CORE = -(-n_win // N_CORES)
    NLOC = W_PER_CORE * P
    N_PAD = N_CORES * NLOC

    perm = np.argsort(dst, kind="stable")
    dst_s = dst[perm]
    src_s = src[perm]
    g_of_e = dst_s // P                       # global window per edge
    wcnt = np.bincount(g_of_e, minlength=N_CORES * W_PER_CORE)
    C = max(1, int(-(-wcnt.max() // P)))
    wstart = np.zeros(len(wcnt) + 1, np.int64)
    np.cumsum(wcnt, out=wstart[1:])
    k = np.arange(len(dst_s)) - wstart[g_of_e]   # rank within window
    core = g_of_e // W_PER_CORE
    col = (g_of_e % W_PER_CORE) * C + k // P     # chunk col within core
    row = k % P

    NCHUNK_COLS = W_PER_CORE * C
    E_PAD = NCHUNK_COLS * P

    cnt_node = np.bincount(dst, minlength=N_PAD).astype(np.float32)
    inv_node = 1.0 / np.maximum(cnt_node, 1.0)

    per_core = []
    ea_s = ea[perm]  # [E, 64] sorted
    for i in range(N_CORES):
        m = core == i
        r, cc = row[m], col[m]
        srcidx = np.zeros((P, NCHUNK_COLS), np.int32)
        srcidx[r, cc] = ((src_s[m] - i * NLOC) % N_PAD).astype(np.int32)
        dstidx = np.zeros((P, NCHUNK_COLS), np.int32)
        dstidx[r, cc] = (dst_s[m] - i * NLOC).astype(np.int32)
        dstloc = np.full((P, NCHUNK_COLS), -1.0, np.float32)
        dstloc[r, cc] = (dst_s[m] % P).astype(np.float32)
        esortT = np.zeros((KE, E_PAD), np.float32)
        esortT[F_EDGE, :] = 1.0
        esortT[0:F_EDGE, cc * P + r] = ea_s[m].T
        invc = inv_node[i * NLOC:(i + 1) * NLOC].reshape(W_PER_CORE, P).T.copy()
        per_core.append(dict(
            srcidx=srcidx, dstidx=dstidx,
            dstloc=dstloc.astype(NP_BF16), esortT=esortT,
            invcnt=np.ascontiguousarray(invc, np.float32),
        ))
    return dict(per_core=per_core, W_PER_CORE=W_PER_CORE, C=C,
                NLOC=NLOC, N_PAD=N_PAD, n_nodes=n_nodes)


def run_two_layers(nc, g, x, params, trace=False):
    """Execute the compiled per-layer NEFF twice. Returns [n_nodes, 128] f32."""
    global LAST_EXEC_NS
    W_PER_CORE, C = g["W_PER_CORE"], g["C"]
    NLOC, N_PAD, n_nodes = g["NLOC"], g["N_PAD"], g["n_nodes"]

    g1, b1 = _fold_bn(params["gamma1"], params["beta1"], params["rmean1"], params["rvar1"])
    g2, b2 = _fold_bn(params["gamma2"], params["beta2"], params["rmean2"], params["rvar2"])
    w_all1, w_ext1 = _layer_weights(params["Wf1"], params["bf1"], params["Ws1"], params["bs1"])
    w_all2, w_ext2 = _layer_weights(params["Wf2"], params["bf2"], params["Ws2"], params["bs2"],
                                    bias_shift=b2)

    x = np.asarray(x, np.float32)
    x_pad = np.zeros((N_PAD, F_NODE), np.float32)
    x_pad[:n_nodes] = x
    xT_glob = np.ascontiguousarray(x_pad.T)

    xr1_glob = np.zeros((N_PAD, F_NODE), np.float32)
    xr1_glob[:n_nodes] = x + b1 + b2

    gb1 = np.broadcast_to(g1, (P, F_NODE)).copy()
    gb2 = np.broadcast_to(g2, (P, F_NODE)).copy()

    def maps_for(layer, xT_g, xr_percore, w_all, w_ext, gb):
        ms = []
        for i in range(N_CORES):
            pc = g["per_core"][i]
            ms.append({
                "xT": np.roll(xT_g, -i * NLOC, axis=1),
                "w_all": w_all, "w_ext": w_ext, "gbc": gb,
                "esortT": pc["esortT"], "srcidx": pc["srcidx"],
                "dstidx": pc["dstidx"], "dstloc": pc["dstloc"],
                "invcnt": pc["invcnt"], "xr": xr_percore[i],
            })
        return ms

    LAST_EXEC_NS = []

    def run_spmd(maps):
        nonlocal trace
        import time as _t
        t0 = _t.time()
        try:
            if trace:
                try:
                    return run_bass_kernel_spmd(
                        nc, maps, core_ids=list(range(N_CORES)), trace=True)
                except ModuleNotFoundError:
                    trace = False  # axon build without NTFF hook
                    import os as _os
                    _os.environ["BASS_NEVER_TRACE"] = "1"
            return run_bass_kernel_spmd(nc, maps, core_ids=list(range(N_CORES)))
        finally:
            LAST_WALL_S.append(_t.time() - t0)

    # layer 1
    xr1_pc = [xr1_glob[i * NLOC:(i + 1) * NLOC] for i in range(N_CORES)]
    m1 = maps_for(1, xT_glob, xr1_pc, w_all1, w_ext1, gb1)
    r1 = run_spmd(m1)
    LAST_EXEC_NS.append(r1.exec_time_ns)

    # stitch: x2b = layer-1 out (= x2 + b2'), already per-core local slices
    x2bT_glob = np.concatenate([r1.results[i]["outT"] for i in range(N_CORES)], axis=1)
    xr2_pc = [r1.results[i]["out"] for i in range(N_CORES)]
    m2 = maps_for(2, x2bT_glob, xr2_pc, w_all2, w_ext2, gb2)
    r2 = run_spmd(m2)
    LAST_EXEC_NS.append(r2.exec_time_ns)

    out = np.concatenate([r2.results[i]["out"] for i in range(N_CORES)], axis=0)
    return out[:n_nodes]


_NC_CACHE = {}


def kernel(x, edge_index, edge_attr,
           Wf1, bf1, Ws1, bs1, gamma1, beta1, rmean1, rvar1,
           Wf2, bf2, Ws2, bs2, gamma2, beta2, rmean2, rvar2):
    import os
    n_nodes = x.shape[0]
    g = preprocess_graph(edge_index, edge_attr, n_nodes)
    key = (g["W_PER_CORE"], g["C"], g["NLOC"], g["N_PAD"])
    if key not in _NC_CACHE:
        _NC_CACHE[key] = build_nc(*key)
    params = dict(Wf1=Wf1, bf1=bf1, Ws1=Ws1, bs1=bs1, gamma1=gamma1,
                  beta1=beta1, rmean1=rmean1, rvar1=rvar1,
                  Wf2=Wf2, bf2=bf2, Ws2=Ws2, bs2=bs2, gamma2=gamma2,
                  beta2=beta2, rmean2=rmean2, rvar2=rvar2)
    trace = bool(os.environ.get("BASS_TRACE"))
    return run_two_layers(_NC_CACHE[key], g, x, params, trace=trace).astype(np.float32)
